# revision 8
# baseline (speedup 1.0000x reference)
"""Mesa-layer memory kernel for Trainium2 (8 NeuronCores, data-parallel over B).

Math: the reference's T-step Sherman-Morrison / discounted-accumulation
recurrence has a closed form,
    R_final = (I + K^T K)^{-1}            (eps term is O(1e-6) relative)
    S_final^T = K^T diag(c) V,   c_t = prod_{s>t} gamma_s
so per memory b the output is
    out_b = Q_b @ (R_b @ S_b^T).
R is computed with Newton-Schulz iterations (pure matmuls; I + K^T K is
well-conditioned here). Iterations run in bf16 with fp32 refinement at the
end; the big T-contracted matmuls and the query readout run in bf16
(validated ~3e-3 max-rel vs the fp32 reference).
c is computed in log space with a free-dim prefix-sum scan.

Each core owns B/8 = 8 independent memories; no cross-core communication.
"""

import numpy as np

B, T, DK, DV, NQ = 64, 2048, 128, 128, 2048
NCORES = 8
BPC = B // NCORES          # memories per core
P = 128                    # partitions
NCH = T // P               # 16 chunks of 128 timesteps
GCLAMP = 1e-30             # gamma clamp before log (exact-0 gammas)

NS_BF = 6                  # Newton-Schulz iterations in bf16
NS_FP = 2                  # fp32 refinement iterations


def build_nc(ns_bf=NS_BF, ns_fp=NS_FP):
    import concourse.mybir as mybir
    import concourse.tile as tile
    from concourse import bacc
    from concourse.masks import make_identity

    fp32 = mybir.dt.float32
    bf16 = mybir.dt.bfloat16
    AF = mybir.ActivationFunctionType
    OP = mybir.AluOpType
    AX = mybir.AxisListType

    # Bacc (not raw Bass): its compile() pass splits multi-sem sync waits to
    # the 1-wait-per-instruction limit the TRN2 encodings require.
    nc = bacc.Bacc(trn_type="TRN2", target_bir_lowering=False, debug=False)
    keys = nc.dram_tensor("keys", [BPC, T, DK], fp32, kind="ExternalInput").ap()
    values = nc.dram_tensor("values", [BPC, T, DV], fp32, kind="ExternalInput").ap()
    gammas = nc.dram_tensor("gammas", [BPC, T], fp32, kind="ExternalInput").ap()
    queries = nc.dram_tensor("queries", [BPC, NQ, DK], fp32, kind="ExternalInput").ap()
    out = nc.dram_tensor("out", [BPC, NQ, DV], fp32, kind="ExternalOutput").ap()

    with tile.TileContext(nc) as tc:
        const = tc.alloc_tile_pool(name="const", bufs=1)
        gam = tc.alloc_tile_pool(name="gam", bufs=1)
        kp = tc.alloc_tile_pool(name="kp", bufs=2)
        vp = tc.alloc_tile_pool(name="vp", bufs=2)
        kvbp = tc.alloc_tile_pool(name="kvbp", bufs=2)
        qp = tc.alloc_tile_pool(name="qp", bufs=2)
        qtp = tc.alloc_tile_pool(name="qtp", bufs=2)
        small = tc.alloc_tile_pool(name="small", bufs=1)
        xs = tc.alloc_tile_pool(name="xs", bufs=2)
        outp = tc.alloc_tile_pool(name="outp", bufs=2)
        ps_as = tc.alloc_tile_pool(name="ps_as", bufs=2, space="PSUM")
        ps_w = tc.alloc_tile_pool(name="ps_w", bufs=4, space="PSUM")

        ident = const.tile([P, P], fp32)
        make_identity(nc, ident)

        # ---- phase 0: per-memory suffix cumprod of gammas (log space) ----
        g_sb = gam.tile([BPC, T], fp32)
        nc.sync.dma_start(g_sb[:], gammas[:, :])
        c8 = gam.tile([BPC, T], fp32)          # also the scan's zero operand
        nc.vector.memset(c8[:], 0.0)
        nc.vector.tensor_scalar_max(g_sb[:], g_sb[:], GCLAMP)
        nc.scalar.activation(g_sb[:], g_sb[:], AF.Ln)
        incl = gam.tile([BPC, T], fp32)
        # joiner: make DVE observe the ACT (Ln) dependency before the scan
        joiner = gam.tile([BPC, 1], fp32)
        nc.vector.tensor_copy(out=joiner[:], in_=g_sb[:, 0:1])
        nc.vector.tensor_tensor_scan(
            incl[:], g_sb[:], c8[:], 0.0, OP.add, OP.add
        )
        # c = exp(total - incl);  total = inclusive sum at t = T-1
        nc.scalar.activation(
            c8[:], incl[:], AF.Exp, bias=incl[:, T - 1 : T], scale=-1.0
        )
        # PE warmup op: absorb the gpsimd(identity) dependency into PE's clock
        ps_warm = ps_w.tile([BPC, BPC], fp32, tag="w", name="ps_warm")
        nc.tensor.transpose(ps_warm[:], ident[:BPC, :BPC], ident[:BPC, :BPC])
        # transpose to [t-within-chunk (partitions), chunk, memory]
        c_t = gam.tile([P, NCH, BPC], fp32)
        for j in range(NCH):
            ps_ct = ps_w.tile([P, BPC], fp32, tag="w", name=f"ps_ct{j}")
            nc.tensor.transpose(ps_ct[:], c8[:, j * P : (j + 1) * P], ident[:BPC, :BPC])
            nc.vector.tensor_copy(out=c_t[:, j, :], in_=ps_ct[:])

        # ---- per-memory state tiles ----
        A_sb = [small.tile([P, P], fp32, tag=f"A{i}", name=f"A{i}") for i in range(BPC)]
        A_bf = [small.tile([P, P], bf16, tag=f"Ab{i}", name=f"Ab{i}") for i in range(BPC)]
        ST_sb = [small.tile([P, P], fp32, tag=f"S{i}", name=f"S{i}") for i in range(BPC)]
        Phi_bf = [small.tile([P, P], bf16, tag=f"Pb{i}", name=f"Phib{i}") for i in range(BPC)]
        rs_sb = [small.tile([P, 1], fp32, tag=f"r{i}", name=f"rs{i}") for i in range(BPC)]

        # ---- phase 1: load K/V; build bf16 [K | diag(c)V]; A and S^T ----
        for i in range(BPC):
            k_sb = kp.tile([P, NCH, DK], fp32, tag="k", name=f"k{i}")
            nc.sync.dma_start(k_sb[:], keys[i].rearrange("(j p) k -> p j k", p=P))
            v_sb = vp.tile([P, NCH, DV], fp32, tag="v", name=f"v{i}")
            nc.sync.dma_start(v_sb[:], values[i].rearrange("(j p) k -> p j k", p=P))

            kvb = kvbp.tile([P, NCH, 2 * P], bf16, tag="kvb", name=f"kvb{i}")
            # K cast fp32->bf16 on the (otherwise idle) GpSimd engine
            nc.gpsimd.tensor_copy(out=kvb[:, :, 0:DK], in_=k_sb[:])
            # V * c (suffix cumprod weights), broadcast along DV, cast to bf16
            nc.vector.tensor_tensor(
                kvb[:, :, DK : 2 * DK],
                v_sb[:],
                c_t[:, :, i, None].to_broadcast((P, NCH, DV)),
                OP.mult,
            )

            ps = ps_as.tile([P, 2 * P], fp32, tag="as", name=f"ps_as{i}")
            for j in range(NCH):
                nc.tensor.matmul(
                    ps[:],
                    kvb[:, j, 0:DK],
                    kvb[:, j, :],
                    start=(j == 0),
                    stop=(j == NCH - 1),
                )

            # A = K^T K + I ; S^T = K^T diag(c) V ; Jacobi-style NS init
            nc.vector.tensor_tensor(A_sb[i][:], ps[:, 0:P], ident[:], OP.add)
            nc.vector.tensor_copy(out=ST_sb[i][:], in_=ps[:, P : 2 * P])
            nc.vector.tensor_copy(out=A_bf[i][:], in_=A_sb[i][:])
            nc.vector.tensor_reduce(
                rs_sb[i][:], A_sb[i][:], AX.X, OP.add, apply_absolute_value=True
            )
            nc.vector.reciprocal(rs_sb[i][:], rs_sb[i][:])

        # ---- phase 2: Newton-Schulz, batched 4 memories per PSUM bank ----
        NG = BPC // 4  # groups of 4 memories
        Xb = []
        for g in range(NG):
            xw = xs.tile([P, 4 * P], bf16, tag=f"Xb{g}", name=f"Xb{g}_0")
            for i in range(4):
                m = 4 * g + i
                nc.scalar.activation(
                    xw[:, i * P : (i + 1) * P], ident[:], AF.Copy,
                    scale=rs_sb[m][:],
                )
            Xb.append(xw)

        for it in range(ns_bf):
            ps_a = []
            for g in range(NG):
                pa = ps_w.tile([P, 4 * P], fp32, tag="w", name=f"pa{it}_{g}")
                for i in range(4):
                    sl = slice(i * P, (i + 1) * P)
                    nc.tensor.matmul(pa[:, sl], A_bf[4 * g + i][:], Xb[g][:, sl])
                ps_a.append(pa)
            ax = []
            for g in range(NG):
                axg = xs.tile([P, 4 * P], bf16, tag=f"ax{g}", name=f"ax{it}_{g}")
                nc.vector.tensor_copy(out=axg[:], in_=ps_a[g][:])
                ax.append(axg)
            ps_b = []
            for g in range(NG):
                pb = ps_w.tile([P, 4 * P], fp32, tag="w", name=f"pb{it}_{g}")
                for i in range(4):
                    sl = slice(i * P, (i + 1) * P)
                    nc.tensor.matmul(pb[:, sl], Xb[g][:, sl], ax[g][:, sl])
                ps_b.append(pb)
            last_bf = it == ns_bf - 1
            for g in range(NG):
                xn = xs.tile(
                    [P, 4 * P], fp32 if last_bf else bf16,
                    tag=f"Xf{g}" if last_bf else f"Xb{g}",
                    name=f"X{g}_{it + 1}",
                )
                nc.vector.scalar_tensor_tensor(
                    xn[:], Xb[g][:], 2.0, ps_b[g][:], OP.mult, OP.subtract
                )
                Xb[g] = xn

        for it in range(ns_fp):
            ps_a = []
            for g in range(NG):
                pa = ps_w.tile([P, 4 * P], fp32, tag="w", name=f"fpa{it}_{g}")
                for i in range(4):
                    sl = slice(i * P, (i + 1) * P)
                    nc.tensor.matmul(pa[:, sl], A_sb[4 * g + i][:], Xb[g][:, sl])
                ps_a.append(pa)
            ax = []
            for g in range(NG):
                axg = xs.tile([P, 4 * P], fp32, tag=f"axf{g}", name=f"fax{it}_{g}")
                nc.vector.tensor_copy(out=axg[:], in_=ps_a[g][:])
                ax.append(axg)
            ps_b = []
            for g in range(NG):
                pb = ps_w.tile([P, 4 * P], fp32, tag="w", name=f"fpb{it}_{g}")
                for i in range(4):
                    sl = slice(i * P, (i + 1) * P)
                    nc.tensor.matmul(pb[:, sl], Xb[g][:, sl], ax[g][:, sl])
                ps_b.append(pb)
            for g in range(NG):
                xn = xs.tile([P, 4 * P], fp32, tag=f"Xf{g}", name=f"Xg{g}_{it + 1}")
                nc.vector.scalar_tensor_tensor(
                    xn[:], Xb[g][:], 2.0, ps_b[g][:], OP.mult, OP.subtract
                )
                Xb[g] = xn

        # ---- phase 3: Phi = R @ S^T (fp32 matmul, bf16 result) ----
        for i in range(BPC):
            g, sl = i // 4, slice((i % 4) * P, (i % 4 + 1) * P)
            ps_phi = ps_w.tile([P, P], fp32, tag="w", name=f"ps_phi{i}")
            nc.tensor.matmul(ps_phi[:], Xb[g][:, sl], ST_sb[i][:])
            nc.vector.tensor_copy(out=Phi_bf[i][:], in_=ps_phi[:])

        # ---- phase 4: out = Q @ Phi; transposes and matmuls packed 4/bank ----
        for i in range(BPC):
            q_sb = qp.tile([P, NCH, DK], fp32, tag="q", name=f"q{i}")
            nc.sync.dma_start(q_sb[:], queries[i].rearrange("(j p) k -> p j k", p=P))
            qt = qtp.tile([P, NCH, P], bf16, tag="qt", name=f"qt{i}")
            for j4 in range(NCH // 4):
                ps_qt = ps_w.tile([P, 4 * P], fp32, tag="w", name=f"ps_qt{i}_{j4}")
                for j in range(4):
                    nc.tensor.transpose(
                        ps_qt[:, j * P : (j + 1) * P], q_sb[:, 4 * j4 + j, :], ident[:]
                    )
                nc.vector.tensor_copy(
                    out=qt[:, 4 * j4 : 4 * j4 + 4, :], in_=ps_qt[:]
                )
            o_sb = outp.tile([P, NCH, DV], fp32, tag="o", name=f"o{i}")
            for j4 in range(NCH // 4):
                ps_o = ps_w.tile([P, 4 * P], fp32, tag="w", name=f"ps_o{i}_{j4}")
                for j in range(4):
                    nc.tensor.matmul(
                        ps_o[:, j * P : (j + 1) * P], qt[:, 4 * j4 + j, :], Phi_bf[i][:]
                    )
                nc.vector.tensor_copy(
                    out=o_sb[:, 4 * j4 : 4 * j4 + 4, :], in_=ps_o[:]
                )
            nc.sync.dma_start(out[i].rearrange("(j p) v -> p j v", p=P), o_sb[:])

        for pool in (ps_w, ps_as, outp, xs, small, qtp, qp, kvbp, vp, kp, gam, const):
            pool.release()

    if not nc.is_finalized():
        nc.finalize()
    return nc


def kernel(**inputs) -> np.ndarray:
    keys = np.ascontiguousarray(inputs["keys"], dtype=np.float32)
    values = np.ascontiguousarray(inputs["values"], dtype=np.float32)
    gammas = np.ascontiguousarray(inputs["gammas"], dtype=np.float32)
    queries = np.ascontiguousarray(inputs["queries"], dtype=np.float32)

    from concourse.bass_utils import run_bass_kernel_spmd

    nc = build_nc()
    in_maps = []
    for m in range(NCORES):
        s = slice(m * BPC, (m + 1) * BPC)
        in_maps.append(
            {
                "keys": keys[s],
                "values": values[s],
                "gammas": gammas[s],
                "queries": queries[s],
            }
        )
    res = run_bass_kernel_spmd(nc, in_maps, core_ids=list(range(NCORES)))
    return np.concatenate([res.results[m]["out"] for m in range(NCORES)], axis=0)
